# revision 6
# baseline (speedup 1.0000x reference)
"""LocalVarianceNet Trainium2 kernel.

Computes E[x^2] - E[x]^2 over a 7x7 circular (wrap-padded) window, per
channel, for x of shape [16, 3, 512, 512] fp32.

Strategy (per 512x512 plane, data-parallel over 8 cores, 6 planes/core):
  Both separable box-filter passes run on the Tensor engine as banded
  matmuls. matmul(out, lhsT=data_chunk, rhs=B_band) computes
  data_chunk^T @ B_band, i.e. it filters the partition dim of the data
  while transposing it. Two such passes compose back to the natural
  orientation:
      pass1: Yt = X^T  B   (vertical sum over rows, output transposed)
      pass2: Z  = Yt^T B   (horizontal sum over cols, natural output)
  B is the 512x512 0/1 circulant band matrix (band width 7, wrap),
  stored as narrow per-128-chunk band slices so each matmul streams only
  ~134 columns. The squared tensor x^2 goes through the same two passes;
  final combine is  var = S2/49 - (S1/49)^2.

  Data is cast to fp16 on the inbound DMA: matmul weight loads (the data
  is the stationary operand) are ~2x faster via FWL for non-fp32 dtypes,
  and all accumulation stays fp32 in PSUM.
"""

import numpy as np

P = 128
HW = 512
PAD = 3  # window 7 -> halo 3
NCH = 4  # 512 / 128 chunks
N_CORES = 8
PLANES_PER_CORE = 6  # (16 images * 3 channels) / 8 cores

# per-k-chunk column range of the band in B (non-wrap part)
_RANGES = []
for _kc in range(NCH):
    _RANGES.append((max(0, _kc * P - PAD), min(HW, _kc * P + P + PAD)))
# widths: 131, 134, 134, 131
_BOFF = [0]
for _lo, _hi in _RANGES:
    _BOFF.append(_BOFF[-1] + (_hi - _lo))
_W0_OFF = _BOFF[NCH]  # wrap piece of k-chunk 0 (writes cols 509..511)
_W3_OFF = _W0_OFF + PAD  # wrap piece of k-chunk 3 (writes cols 0..2)
_BMAT_COLS = _W3_OFF + PAD  # 536


def _make_bmat(np_dtype):
    """Band slices of the circulant 0/1 matrix, concatenated [128, 536]."""
    B = np.zeros((HW, HW), np.float32)
    idx = np.arange(HW)
    for d in range(-PAD, PAD + 1):
        B[idx, (idx + d) % HW] = 1.0
    parts = [B[kc * P : (kc + 1) * P, lo:hi] for kc, (lo, hi) in enumerate(_RANGES)]
    parts.append(B[0:P, HW - PAD : HW])  # w0: k-chunk 0 -> cols 509..511
    parts.append(B[(NCH - 1) * P : HW, 0:PAD])  # w3: k-chunk 3 -> cols 0..2
    return np.ascontiguousarray(np.concatenate(parts, axis=1).astype(np_dtype))


def _band_pass(nc, ps, lhsT_of, bm):
    """Accumulate the circular 7-wide band filter into psum ps [128, 512].

    ps[m, i] = sum_k lhsT_of(chunk(k))[klocal, m] * B[k, i]

    Matmuls are split so each instruction's PSUM region is either fully
    first-write or fully accumulate (per-element has_written semantics are
    modeled at instruction granularity by CoreSim). Program order per bank:
      kc0 main [0,131) start=True, then for kc=1..3 a 6-col fringe
      (accumulates into the previous main's tail) + a main (first-write),
      then the two wrap corners (accumulate).
    """
    OV = 2 * PAD  # 6-col overlap between adjacent chunk bands
    seq = []
    # kc0 main: cols [0, 131)
    lo0, hi0 = _RANGES[0]
    seq.append((lhsT_of(0), bm[:, _BOFF[0] : _BOFF[0] + (hi0 - lo0)], ps[:, lo0:hi0]))
    for kc in range(1, NCH):
        lo, hi = _RANGES[kc]
        b0 = _BOFF[kc]
        # fringe: cols [lo, lo+6) overlap the previous main -> accumulate
        seq.append((lhsT_of(kc), bm[:, b0 : b0 + OV], ps[:, lo : lo + OV]))
        # main: cols [lo+6, hi) -> first write
        seq.append((lhsT_of(kc), bm[:, b0 + OV : b0 + (hi - lo)], ps[:, lo + OV : hi]))
    # wrap corners (both accumulate into already-written cols)
    seq.append((lhsT_of(NCH - 1), bm[:, _W3_OFF : _W3_OFF + PAD], ps[:, 0:PAD]))
    seq.append((lhsT_of(0), bm[:, _W0_OFF : _W0_OFF + PAD], ps[:, HW - PAD : HW]))
    n = len(seq)
    for i, (lh, rh, out) in enumerate(seq):
        nc.tensor.matmul(out, lh, rh, start=(i == 0), stop=(i == n - 1))


def build(n_planes=PLANES_PER_CORE):
    import concourse.mybir as mybir
    from concourse import bacc
    from concourse.tile import TileContext

    f16 = mybir.dt.float16
    f32 = mybir.dt.float32
    SQ = mybir.ActivationFunctionType.Square
    MUL = mybir.AluOpType.mult
    SUB = mybir.AluOpType.subtract
    INV = 1.0 / 49.0

    nc = bacc.Bacc("TRN2", target_bir_lowering=False)
    x_d = nc.declare_dram_parameter("x", [n_planes, HW, HW], f32, isOutput=False)
    b_d = nc.declare_dram_parameter("bmat", [P, _BMAT_COLS], f16, isOutput=False)
    o_d = nc.declare_dram_parameter("out", [n_planes, HW, HW], f32, isOutput=True)

    with TileContext(nc) as tc:
        with (
            tc.tile_pool(name="const", bufs=1) as constp,
            tc.tile_pool(name="xin", bufs=3) as xinp,
            tc.tile_pool(name="xsq", bufs=3) as xsqp,
            tc.tile_pool(name="yt", bufs=4) as ytp,
            tc.tile_pool(name="tsq", bufs=3) as tsqp,
            tc.tile_pool(name="outp", bufs=3) as outpp,
            tc.tile_pool(name="psA", bufs=3, space="PSUM") as psAp,
            tc.tile_pool(name="psZ", bufs=2, space="PSUM") as psZp,
        ):
            bm_t = constp.tile([P, _BMAT_COLS], f16)
            nc.sync.dma_start(out=bm_t[:], in_=b_d[:, :])
            bm = bm_t[:]

            for p in range(n_planes):
                xin = xinp.tile([P, NCH, HW], f16)
                nc.gpsimd.dma_start(
                    out=xin[:], in_=x_d[p].rearrange("(kc q) c -> q kc c", q=P)
                )
                xsq = xsqp.tile([P, NCH, HW], f16)
                nc.gpsimd.tensor_mul(out=xsq[:], in0=xin[:], in1=xin[:])

                yts = {}
                for t, src in (("x", xin), ("x2", xsq)):
                    yt = ytp.tile([P, NCH, HW], f16, tag=f"yt_{t}")
                    yts[t] = yt
                    for jc in range(NCH):
                        ps = psAp.tile([P, HW], f32)
                        _band_pass(
                            nc, ps[:], lambda kc: src[:, kc, jc * P : (jc + 1) * P], bm
                        )
                        if t == "x":
                            nc.scalar.copy(out=yt[:, jc, :], in_=ps[:])
                        else:
                            nc.vector.tensor_copy(out=yt[:, jc, :], in_=ps[:])

                outt = outpp.tile([P, NCH, HW], f32)
                for ic in range(NCH):
                    ps1 = psZp.tile([P, HW], f32, tag="s1")
                    _band_pass(
                        nc, ps1[:], lambda jc: yts["x"][:, jc, ic * P : (ic + 1) * P], bm
                    )
                    ps2 = psZp.tile([P, HW], f32, tag="s2")
                    _band_pass(
                        nc, ps2[:], lambda jc: yts["x2"][:, jc, ic * P : (ic + 1) * P], bm
                    )
                    ts_ = tsqp.tile([P, HW], f32)
                    nc.scalar.activation(out=ts_[:], in_=ps1[:], func=SQ, scale=INV)
                    nc.vector.scalar_tensor_tensor(
                        out=outt[:, ic, :],
                        in0=ps2[:],
                        scalar=INV,
                        in1=ts_[:],
                        op0=MUL,
                        op1=SUB,
                    )
                nc.sync.dma_start(
                    out=o_d[p].rearrange("(ic q) c -> q ic c", q=P), in_=outt[:]
                )
    nc.compile()
    return nc


_CACHED = {}


def _get_nc(n_planes=PLANES_PER_CORE):
    if n_planes not in _CACHED:
        _CACHED[n_planes] = build(n_planes)
    return _CACHED[n_planes]


def kernel(x: np.ndarray) -> np.ndarray:
    from concourse.bass_utils import run_bass_kernel_spmd

    N, C, H, W = x.shape
    assert (H, W) == (HW, HW), (H, W)
    planes = np.ascontiguousarray(x.reshape(N * C, H, W).astype(np.float32))
    total = N * C
    per_core = total // N_CORES
    assert per_core == PLANES_PER_CORE, (total, N_CORES)

    bmat = _make_bmat(np.float16)
    nc = _get_nc(per_core)

    in_maps = [
        {
            "x": np.ascontiguousarray(planes[i * per_core : (i + 1) * per_core]),
            "bmat": bmat,
        }
        for i in range(N_CORES)
    ]
    res = run_bass_kernel_spmd(nc, in_maps, list(range(N_CORES)))
    out = np.concatenate([r["out"] for r in res.results], axis=0)
    return out.reshape(N, C, H, W).astype(np.float32)
